# revision 9
# baseline (speedup 1.0000x reference)
"""Trainium2 Bass kernel for the adaptive-span Mamba block (moe_routing).

Strategy
--------
Host side (index/marshalling work only): compute the span-group routing
(|avg_span - s| < 1.5 per group, ~23% of (position, group) pairs are active),
the flow-guided window centers, bilinear corner coords + fractional weights.
Pack, per core (64 of the 512 position-streams each), the gathered integer
image patches, per-pair blend-weight rows, and pair->position scatter
matrices as *input tensors*. The program is identical for all 8 cores (SPMD);
its shape depends only on the per-group pair capacity `caps` (bucketed so the
compiled NEFF is reusable across calls).

Device side (all the FLOPs): bilinear blend of patches (DVE, bf16),
RMSNorm stats via ones-matmul (PE) + row rsqrt, Win/Wout matmuls (PE bf16,
channel-major, PSUM token tiles of 512), the Mamba linear recurrence via the
native DVE tensor_tensor_scan (fp32 decay tiles with a=0 reset columns
between windows), depth-2 evaluated only up to each window's center token,
and the final 1x1 aggregators as PE matmuls with a pair->position scatter
matmul accumulating (64, C) output slices in PSUM.

Exactness notes: only tokens 0..center feed the outputs (the scan is causal
and only the center token is read), so sequences are truncated to the
(half+1) x s leading rectangle of each window. Masked-out (position, group)
pairs contribute exactly 0; positions with no active group produce the bias.
"""

import os
import sys
import math
import numpy as np

for _p in ("/opt/trn_rl_repo",):
    if _p not in sys.path:
        sys.path.insert(0, _p)

import concourse.bass as bass
import concourse.bacc as bacc
import concourse.tile as tile
import concourse.mybir as mybir
from concourse.bass_utils import run_bass_kernel_spmd

BF16 = mybir.dt.bfloat16
F32 = mybir.dt.float32
NPBF16 = mybir.dt.np(mybir.dt.bfloat16)
ALU = mybir.AluOpType
ACTF = mybir.ActivationFunctionType

SPAN_GROUPS = (5, 7, 9, 11, 15)
TOL = 1.5
DEPTH = 2
B, C, DG, H, W = 1, 256, 64, 16, 16
G = len(SPAN_GROUPS)
N_CORES = 8
P_CORE = 64          # position-streams per core (2 streams x 256 positions / 8)
PAD = 7              # padded image: index = coord + PAD, size 31x31
IMG = H + PAD + 8    # 31
EPS = 1e-6
TOKTILE = 512        # PSUM token tile (moving dim per matmul)

# per-group geometry
HALF = [s // 2 for s in SPAN_GROUPS]
PSZ_R = [h + 2 for h in HALF]            # patch rows
PSZ_C = [s + 1 for s in SPAN_GROUPS]     # patch cols
T_R = [h + 1 for h in HALF]              # window rect rows (tokens kept)
T_G = [(h + 1) * s for h, s in zip(HALF, SPAN_GROUPS)]   # tokens per window
CENTER = [h * s + h for h, s in zip(HALF, SPAN_GROUPS)]  # center token index


# --------------------------------------------------------------------------
# host-side routing / packing
# --------------------------------------------------------------------------

def _routing(inputs):
    """Returns per-core pair lists: pairs[core][g] = list of
    (local_pos, y0, x0, fy, fx)."""
    pairs = [[[] for _ in range(G)] for _ in range(N_CORES)]
    for s_id in range(2):
        flow = np.asarray(inputs[f"flow_map_{s_id}"])[0]          # (16,16,4)
        sx = np.asarray(inputs[f"adaptive_spans_x_{s_id}"])[0]
        sy = np.asarray(inputs[f"adaptive_spans_y_{s_id}"])[0]
        yy, xx = np.meshgrid(np.arange(H, dtype=np.float32),
                             np.arange(W, dtype=np.float32), indexing="ij")
        cy = np.clip(yy + flow[..., 1], 0, H - 1).astype(np.float32)
        cx = np.clip(xx + flow[..., 0], 0, W - 1).astype(np.float32)
        y0 = np.floor(cy).astype(np.int64)
        x0 = np.floor(cx).astype(np.int64)
        fy = (cy - y0).astype(np.float32)
        fx = (cx - x0).astype(np.float32)
        avg = ((sx + sy) * 0.5).astype(np.float32)
        for n in range(H * W):
            yi, xi = divmod(n, W)
            p_glob = s_id * H * W + n
            core, local = p_glob % N_CORES, p_glob // N_CORES
            for g, s in enumerate(SPAN_GROUPS):
                if abs(float(avg[yi, xi]) - s) < TOL:
                    pairs[core][g].append(
                        (local, s_id, int(y0[yi, xi]), int(x0[yi, xi]),
                         float(fy[yi, xi]), float(fx[yi, xi])))
    return pairs


def _caps_from_pairs(pairs):
    caps = []
    for g in range(G):
        m = max(len(pairs[c][g]) for c in range(N_CORES))
        caps.append(max(4, int(math.ceil(m / 4.0)) * 4))
    return tuple(caps)


def _pack_core(inputs, pairs_core, caps, core):
    """Build the per-core input arrays."""
    imgs = []
    for s_id in range(2):
        feat = np.asarray(inputs[f"feat_match_{s_id}"])[0]       # (C,16,16)
        img = np.zeros((C, IMG, IMG), np.float32)
        img[:, PAD:PAD + H, PAD:PAD + W] = feat
        imgs.append(img)
    out = {}
    for g, s in enumerate(SPAN_GROUPS):
        cap, half = caps[g], HALF[g]
        pr, pc, tg = PSZ_R[g], PSZ_C[g], T_G[g]
        psz = pr * pc
        pat = np.zeros((128, 2, cap, pr, pc), np.float32)
        wp = np.zeros((4, cap, half + 1, s), np.float32)
        E = np.zeros((cap + (1 if g == 0 else 0), P_CORE), np.float32)
        for i, (local, s_id, y0, x0, fy, fx) in enumerate(pairs_core[g]):
            r0 = y0 - half + PAD
            c0 = x0 - half + PAD
            patch = imgs[s_id][:, r0:r0 + pr, c0:c0 + pc]        # (C,pr,pc)
            pat[:, 0, i] = patch[:128]
            pat[:, 1, i] = patch[128:]
            wp[0, i] = (1 - fy) * (1 - fx)
            wp[1, i] = (1 - fy) * fx
            wp[2, i] = fy * (1 - fx)
            wp[3, i] = fy * fx
            E[i, local] = 1.0
        if g == 0:
            E[cap, :] = 1.0  # bias row
        out[f"patches_{g}"] = np.ascontiguousarray(
            pat.reshape(128, 2 * cap * psz)).astype(NPBF16)
        wpb = np.broadcast_to(wp.reshape(1, 4, cap * tg),
                              (128, 4, cap * tg))
        out[f"wpat_{g}"] = np.ascontiguousarray(
            wpb.reshape(128, 4 * cap * tg)).astype(NPBF16)
        out[f"emat_{g}"] = np.ascontiguousarray(E).astype(NPBF16)
    return out


def _pack_weights(inputs):
    """Shared (replicated) weight arrays, partition-major for single DMAs."""
    Win = np.asarray(inputs["Win"], np.float32)       # (G,2,C,2C)
    alog = np.asarray(inputs["alog"], np.float32)     # (G,2,C)
    Wout = np.asarray(inputs["Wout"], np.float32)     # (G,2,C,C)
    Wgeom = np.asarray(inputs["Wgeom"], np.float32)   # (G,2,C,DG)
    gamma = np.asarray(inputs["gamma"], np.float32)   # (G,2,C)
    Wagg = np.asarray(inputs["Wagg"], np.float32)     # (G*C, C)
    bagg = np.asarray(inputs["bagg"], np.float32)     # (C,)
    Wagg_g = np.asarray(inputs["Wagg_g"], np.float32)  # (G*DG, DG)
    bagg_g = np.asarray(inputs["bagg_g"], np.float32)  # (DG,)

    Winp = Win * gamma[..., None]                     # fold gamma
    # (128, G*2*2*512): [p, ((g*2+d)*2+k)*512 + m] = Winp[g,d,k*128+p,m]
    wi = Winp.reshape(G, 2, 2, 128, 2 * C).transpose(3, 0, 1, 2, 4)
    wo = Wout.reshape(G, 2, 2, 128, C).transpose(3, 0, 1, 2, 4)
    wg = Wgeom[:, 1].reshape(G, 2, 128, DG).transpose(2, 0, 1, 3)   # d=1 only
    wa = Wagg.reshape(G, 2, 128, C).transpose(2, 0, 1, 3)
    wag = Wagg_g.reshape(G, DG, DG).transpose(1, 0, 2)              # (64,G,64)
    a = 1.0 / (1.0 + np.exp(-alog))                   # sigmoid, fp32
    av = a.reshape(G, 2, 2, 128).transpose(3, 0, 1, 2)              # (128,G,2,2)
    out = {
        "w_in": np.ascontiguousarray(wi.reshape(128, -1)).astype(NPBF16),
        "w_out": np.ascontiguousarray(wo.reshape(128, -1)).astype(NPBF16),
        "w_geom": np.ascontiguousarray(wg.reshape(128, -1)).astype(NPBF16),
        "w_agg": np.ascontiguousarray(wa.reshape(128, -1)).astype(NPBF16),
        "w_agg_g": np.ascontiguousarray(wag.reshape(DG, -1)).astype(NPBF16),
        "avals": np.ascontiguousarray(av.reshape(128, -1)),
        "bagg_row": bagg.reshape(1, C).astype(NPBF16),
        "bagg_g_row": bagg_g.reshape(1, DG).astype(NPBF16),
        "eye": np.eye(128, dtype=NPBF16),
    }
    return out


# --------------------------------------------------------------------------
# device program
# --------------------------------------------------------------------------

def build_program(caps):
    nc = bacc.Bacc("TRN2", target_bir_lowering=False, debug=False,
                   num_devices=N_CORES)

    din = {}
    din["w_in"] = nc.dram_tensor("w_in", [128, G * 2 * 2 * 2 * C], BF16,
                                 kind="ExternalInput").ap()
    din["w_out"] = nc.dram_tensor("w_out", [128, G * 2 * 2 * C], BF16,
                                  kind="ExternalInput").ap()
    din["w_geom"] = nc.dram_tensor("w_geom", [128, G * 2 * DG], BF16,
                                   kind="ExternalInput").ap()
    din["w_agg"] = nc.dram_tensor("w_agg", [128, G * 2 * C], BF16,
                                  kind="ExternalInput").ap()
    din["w_agg_g"] = nc.dram_tensor("w_agg_g", [DG, G * DG], BF16,
                                    kind="ExternalInput").ap()
    din["avals"] = nc.dram_tensor("avals", [128, G * 2 * 2], F32,
                                  kind="ExternalInput").ap()
    din["bagg_row"] = nc.dram_tensor("bagg_row", [1, C], BF16,
                                     kind="ExternalInput").ap()
    din["bagg_g_row"] = nc.dram_tensor("bagg_g_row", [1, DG], BF16,
                                       kind="ExternalInput").ap()
    din["eye"] = nc.dram_tensor("eye", [128, 128], BF16,
                                kind="ExternalInput").ap()
    for g in range(G):
        cap = caps[g]
        psz = PSZ_R[g] * PSZ_C[g]
        din[f"patches_{g}"] = nc.dram_tensor(
            f"patches_{g}", [128, 2 * cap * psz], BF16,
            kind="ExternalInput").ap()
        din[f"wpat_{g}"] = nc.dram_tensor(
            f"wpat_{g}", [128, 4 * cap * T_G[g]], BF16,
            kind="ExternalInput").ap()
        din[f"emat_{g}"] = nc.dram_tensor(
            f"emat_{g}", [cap + (1 if g == 0 else 0), P_CORE], BF16,
            kind="ExternalInput").ap()

    m_out = nc.dram_tensor("m_slice", [P_CORE, C], F32,
                           kind="ExternalOutput").ap()
    g_out = nc.dram_tensor("g_slice", [P_CORE, DG], F32,
                           kind="ExternalOutput").ap()

    with tile.TileContext(nc) as tc:
        _emit(tc, nc, din, m_out, g_out, caps)
    nc.compile()
    return nc


def _emit(tc, nc, din, m_out, g_out, caps):
    from contextlib import ExitStack
    ctx = ExitStack()
    cpool = ctx.enter_context(tc.tile_pool(name="const", bufs=1))
    gpool = ctx.enter_context(tc.tile_pool(name="grp", bufs=1))
    ppool = ctx.enter_context(tc.tile_pool(name="patch", bufs=1))
    mm_ps = ctx.enter_context(tc.tile_pool(name="mmps", bufs=2, space="PSUM"))
    row_ps = ctx.enter_context(tc.tile_pool(name="rowps", bufs=1, space="PSUM"))
    sm_ps = ctx.enter_context(tc.tile_pool(name="smps", bufs=1, space="PSUM"))
    acc_ps = ctx.enter_context(tc.tile_pool(name="accps", bufs=1, space="PSUM"))

    # ---- constants in SBUF
    w_in = cpool.tile([128, G * 2 * 2 * 2 * C], BF16)
    nc.sync.dma_start(out=w_in[:], in_=din["w_in"][:])
    w_outw = cpool.tile([128, G * 2 * 2 * C], BF16)
    nc.sync.dma_start(out=w_outw[:], in_=din["w_out"][:])
    w_geom = cpool.tile([128, G * 2 * DG], BF16)
    nc.sync.dma_start(out=w_geom[:], in_=din["w_geom"][:])
    w_agg = cpool.tile([128, G * 2 * C], BF16)
    nc.sync.dma_start(out=w_agg[:], in_=din["w_agg"][:])
    w_agg_g = cpool.tile([DG, G * DG], BF16)
    nc.sync.dma_start(out=w_agg_g[:], in_=din["w_agg_g"][:])
    avals = cpool.tile([128, G * 2 * 2], F32)
    nc.sync.dma_start(out=avals[:], in_=din["avals"][:])
    eye = cpool.tile([128, 128], BF16)
    nc.sync.dma_start(out=eye[:], in_=din["eye"][:])
    ones_col = cpool.tile([128, 1], BF16)
    nc.vector.memset(ones_col[:], 1.0)
    ones_row = cpool.tile([1, 128], BF16)
    nc.vector.memset(ones_row[:], 1.0)

    # lhsT slice helpers into packed weight tiles
    def win_lhsT(g, d, k, m):           # (128,128) chunk of Win' [K=c,M=2C]
        base = ((g * 2 + d) * 2 + k) * (2 * C)
        return w_in[:, base + m * 128: base + (m + 1) * 128]

    def wout_lhsT(g, d, k, m):
        base = ((g * 2 + d) * 2 + k) * C
        return w_outw[:, base + m * 128: base + (m + 1) * 128]

    def wgeom_lhsT(g, k):
        base = (g * 2 + k) * DG
        return w_geom[:, base: base + DG]

    def wagg_lhsT(g, k, m):
        base = (g * 2 + k) * C
        return w_agg[:, base + m * 128: base + (m + 1) * 128]

    def waggg_lhsT(g):
        return w_agg_g[:, g * DG: (g + 1) * DG]

    # SBUF accumulators for the final aggregation
    m_acc = cpool.tile([P_CORE, C], F32)
    g_acc = cpool.tile([P_CORE, DG], F32)

    for g in range(G):
        s = SPAN_GROUPS[g]
        cap, half = caps[g], HALF[g]
        pr, pc, tr = PSZ_R[g], PSZ_C[g], T_R[g]
        tg, cen = T_G[g], CENTER[g]
        X = cap * tg
        capg = cap + (1 if g == 0 else 0)

        # ---- load patches / weight rows / E
        patch = ppool.tile([128, 2, cap, pr, pc], BF16, tag="patch")
        nc.sync.dma_start(out=patch[:].rearrange("p a b c d -> p (a b c d)"),
                          in_=din[f"patches_{g}"][:])
        wpat = gpool.tile([128, 4, X], BF16, tag="wpat")
        nc.sync.dma_start(out=wpat[:].rearrange("p a b -> p (a b)"),
                          in_=din[f"wpat_{g}"][:])
        emat = ppool.tile([capg, P_CORE], BF16, tag="emat")
        nc.sync.dma_start(out=emat[:], in_=din[f"emat_{g}"][:])

        # ---- zones: 1.0 with 0.0 in col 0 of every pair (shared by depths)
        zones = gpool.tile([128, cap, tg], BF16, tag="zones")
        nc.gpsimd.memset(zones[:].rearrange("p a b -> p (a b)"), 1.0)
        nc.gpsimd.memset(zones[:, :, 0], 0.0)

        # ---- bilinear blend -> x0 (128, 2, cap, tr, s)
        x0 = gpool.tile([128, 2, cap, tr, s], BF16, tag="x0")
        tmp = gpool.tile([128, cap, tr, s], BF16, tag="btmp")
        for ch in range(2):
            xv = x0[:, ch]
            taps = ((0, 0, 0), (1, 0, 1), (2, 1, 0), (3, 1, 1))
            for wi, dy, dx in taps[:1]:
                nc.vector.tensor_tensor(
                    out=xv[:],
                    in0=patch[:, ch, :, dy:dy + tr, dx:dx + s],
                    in1=wpat[:, wi, :]
                    .rearrange("q (a b c) -> q a b c", a=cap, b=tr),
                    op=ALU.mult)
            for wi, dy, dx in taps[1:]:
                nc.vector.tensor_tensor(
                    out=tmp[:],
                    in0=patch[:, ch, :, dy:dy + tr, dx:dx + s],
                    in1=wpat[:, wi, :]
                    .rearrange("q (a b c) -> q a b c", a=cap, b=tr),
                    op=ALU.mult)
                nc.vector.tensor_tensor(
                    out=xv[:].rearrange("p a b c -> p (a b c)"),
                    in0=xv[:].rearrange("p a b c -> p (a b c)"),
                    in1=tmp[:].rearrange("p a b c -> p (a b c)"),
                    op=ALU.add)

        x1 = gpool.tile([128, 2, cap, tr, s], BF16, tag="x1")
        u_t = gpool.tile([128, 2, X], BF16, tag="u")
        z_t = gpool.tile([128, 2, X], BF16, tag="z")
        h_t = gpool.tile([128, 2, X], BF16, tag="h")
        y_t = gpool.tile([128, 2, X], BF16, tag="y")
        x2 = gpool.tile([128, 2, X], BF16, tag="x2")
        xn = gpool.tile([128, 2, X], BF16, tag="xn")
        a_t = gpool.tile([128, 2, X], F32, tag="a_t")
        v_row = gpool.tile([1, X], F32, tag="v_row")
        r_row = gpool.tile([1, X], F32, tag="r_row")
        rs_row = gpool.tile([1, X], BF16, tag="rs_row")
        rs_sb = gpool.tile([128, X], BF16, tag="rs_sb")

        ntt = (X + TOKTILE - 1) // TOKTILE

        for d in range(DEPTH):
            xcur = x0 if d == 0 else x1
            xflat = xcur[:].rearrange("p a b c d -> p a (b c d)")

            # x^2 (ACT) then column-sum via ones-matmul, then rs row
            for ch in range(2):
                nc.scalar.activation(out=x2[:, ch], in_=xflat[:, ch],
                                     func=ACTF.Square)
            for t0 in range(0, X, TOKTILE):
                tl = min(TOKTILE, X - t0)
                s_ps = row_ps.tile([1, TOKTILE], F32, tag="s_ps")
                for ch in range(2):
                    nc.tensor.matmul(s_ps[:, :tl], ones_col[:],
                                     x2[:, ch, t0:t0 + tl],
                                     start=(ch == 0), stop=(ch == 1))
                nc.scalar.activation(out=v_row[:, t0:t0 + tl],
                                     in_=s_ps[:, :tl], func=ACTF.Copy,
                                     bias=EPS, scale=1.0 / C)
            nc.vector.reciprocal(out=r_row[:], in_=v_row[:])
            nc.scalar.activation(out=rs_row[:], in_=r_row[:], func=ACTF.Sqrt)
            for t0 in range(0, X, TOKTILE):
                tl = min(TOKTILE, X - t0)
                bc_ps = row_ps.tile([128, TOKTILE], F32, tag="bc_ps")
                nc.tensor.matmul(bc_ps[:, :tl], ones_row[:],
                                 rs_row[:, t0:t0 + tl], start=True, stop=True)
                nc.scalar.activation(out=rs_sb[:, t0:t0 + tl],
                                     in_=bc_ps[:, :tl], func=ACTF.Copy)

            # decay tile for the scan: zones * a  (fp32)
            for ch in range(2):
                av = avals[:, (g * 2 + d) * 2 + ch: (g * 2 + d) * 2 + ch + 1]
                zf = zones[:].rearrange("p a b -> p (a b)")
                if d == 0:
                    nc.scalar.activation(out=a_t[:, ch], in_=zf,
                                         func=ACTF.Copy, scale=av)
                else:
                    nc.vector.tensor_scalar_mul(out=a_t[:, ch], in0=zf,
                                                scalar1=av)

            if d == 0:
                # xn = x * rs ; u,z = xn @ Win'
                for ch in range(2):
                    nc.vector.tensor_tensor(
                        out=xn[:, ch], in0=xflat[:, ch],
                        in1=rs_sb[:], op=ALU.mult)
                for t0 in range(0, X, TOKTILE):
                    tl = min(TOKTILE, X - t0)
                    for m in range(4):  # 0,1 -> u ; 2,3 -> z
                        ps = mm_ps.tile([128, TOKTILE], F32, tag="mmps")
                        for k in range(2):
                            nc.tensor.matmul(ps[:, :tl], win_lhsT(g, d, k, m),
                                             xn[:, k, t0:t0 + tl],
                                             start=(k == 0), stop=(k == 1))
                        dst = u_t if m < 2 else z_t
                        if m < 2:
                            nc.vector.tensor_copy(
                                dst[:, m % 2, t0:t0 + tl], ps[:, :tl])
                        else:
                            nc.scalar.activation(
                                out=dst[:, m % 2, t0:t0 + tl], in_=ps[:, :tl],
                                func=ACTF.Copy)
            else:
                # u = rs * (x1 @ Wu')   (post-scaled)
                for t0 in range(0, X, TOKTILE):
                    tl = min(TOKTILE, X - t0)
                    for m in range(2):
                        ps = mm_ps.tile([128, TOKTILE], F32, tag="mmps")
                        for k in range(2):
                            nc.tensor.matmul(ps[:, :tl], win_lhsT(g, d, k, m),
                                             xflat[:, k, t0:t0 + tl],
                                             start=(k == 0), stop=(k == 1))
                        nc.vector.tensor_tensor(
                            out=u_t[:, m, t0:t0 + tl], in0=ps[:, :tl],
                            in1=rs_sb[:, t0:t0 + tl], op=ALU.mult)

            # the scan: h = a*h + u along each pair's token run
            for ch in range(2):
                nc.vector.tensor_tensor_scan(
                    out=h_t[:, ch], data0=a_t[:, ch], data1=u_t[:, ch],
                    initial=0.0, op0=ALU.mult, op1=ALU.add)

            if d == 0:
                for ch in range(2):
                    nc.vector.tensor_tensor(out=y_t[:, ch], in0=h_t[:, ch],
                                            in1=z_t[:, ch], op=ALU.mult)
                    nc.scalar.activation(out=z_t[:, ch], in_=z_t[:, ch],
                                         func=ACTF.Sigmoid)
                    nc.vector.tensor_tensor(out=y_t[:, ch], in0=y_t[:, ch],
                                            in1=z_t[:, ch], op=ALU.mult)
                # x1 = y @ Wout' + x0
                x1f = x1[:].rearrange("p a b c d -> p a (b c d)")
                x0f = x0[:].rearrange("p a b c d -> p a (b c d)")
                for t0 in range(0, X, TOKTILE):
                    tl = min(TOKTILE, X - t0)
                    for m in range(2):
                        ps = mm_ps.tile([128, TOKTILE], F32, tag="mmps")
                        for k in range(2):
                            nc.tensor.matmul(ps[:, :tl], wout_lhsT(g, d, k, m),
                                             y_t[:, k, t0:t0 + tl],
                                             start=(k == 0), stop=(k == 1))
                        nc.vector.tensor_tensor(
                            out=x1f[:, m, t0:t0 + tl], in0=ps[:, :tl],
                            in1=x0f[:, m, t0:t0 + tl], op=ALU.add)

        # ---- depth-2 center-only tail
        x1f3 = x1[:].rearrange("p a b c d -> p a (c d) b")  # wrong path safety
        # center views: (128, cap) strided by tg
        h_c = [h_t[:, ch].rearrange("p (a b) -> p a b", a=cap)[:, :, cen]
               for ch in range(2)]
        x1_c = [x1[:].rearrange("p a b c d -> p a b (c d)")[:, ch, :, cen]
                for ch in range(2)]
        rs_c = rs_sb[:].rearrange("q (a b) -> q a b", a=cap)[:, :, cen]

        z_c = gpool.tile([128, 2, cap], BF16, tag="z_c")
        y_c = gpool.tile([128, 2, cap], BF16, tag="y_c")
        mc = gpool.tile([128, 2, cap], BF16, tag="mc")
        gc = gpool.tile([DG, cap], BF16, tag="gc")

        for m in range(2):   # z chunks are M-chunks 2,3 of Win'
            ps = sm_ps.tile([128, cap], F32, tag="smps")
            for k in range(2):
                nc.tensor.matmul(ps[:], win_lhsT(g, 1, k, 2 + m),
                                 x1_c[k], start=(k == 0), stop=(k == 1))
            nc.vector.tensor_tensor(out=z_c[:, m], in0=ps[:],
                                    in1=rs_c, op=ALU.mult)
        for ch in range(2):
            nc.vector.tensor_tensor(out=y_c[:, ch], in0=h_c[ch],
                                    in1=z_c[:, ch], op=ALU.mult)
            nc.scalar.activation(out=z_c[:, ch], in_=z_c[:, ch],
                                 func=ACTF.Sigmoid)
            nc.vector.tensor_tensor(out=y_c[:, ch], in0=y_c[:, ch],
                                    in1=z_c[:, ch], op=ALU.mult)
        # m_center = y_c @ Wout'[d=1] + x1_center ; g_center = y_c @ Wgeom
        for m in range(2):
            ps = sm_ps.tile([128, cap], F32, tag="smps")
            for k in range(2):
                nc.tensor.matmul(ps[:], wout_lhsT(g, 1, k, m),
                                 y_c[:, k], start=(k == 0), stop=(k == 1))
            nc.vector.tensor_tensor(out=mc[:, m], in0=ps[:], in1=x1_c[m],
                                    op=ALU.add)
        ps = sm_ps.tile([DG, cap], F32, tag="gps")
        for k in range(2):
            nc.tensor.matmul(ps[:], wgeom_lhsT(g, k), y_c[:, k],
                             start=(k == 0), stop=(k == 1))
        nc.scalar.activation(out=gc[:], in_=ps[:], func=ACTF.Copy)

        # ---- aggregation: pcon = Wagg_g^T @ mc ; transpose; scatter-matmul
        pcon = gpool.tile([128, 2, cap], BF16, tag="pcon")
        for m in range(2):
            ps = sm_ps.tile([128, cap], F32, tag="smps")
            for k in range(2):
                nc.tensor.matmul(ps[:], wagg_lhsT(g, k, m), mc[:, k],
                                 start=(k == 0), stop=(k == 1))
            nc.scalar.activation(out=pcon[:, m], in_=ps[:], func=ACTF.Copy)
        gcon = gpool.tile([DG, cap], BF16, tag="gcon")
        ps = sm_ps.tile([DG, cap], F32, tag="gps")
        nc.tensor.matmul(ps[:], waggg_lhsT(g), gc[:], start=True, stop=True)
        nc.scalar.activation(out=gcon[:], in_=ps[:], func=ACTF.Copy)

        pairT = gpool.tile([capg, C], BF16, tag="pairT")
        gpairT = gpool.tile([capg, DG], BF16, tag="gpairT")
        for m in range(2):
            tp = sm_ps.tile([cap, 128], BF16, tag="smps")
            nc.tensor.transpose(tp[:], pcon[:, m], eye[:])
            nc.vector.tensor_copy(pairT[:cap, m * 128:(m + 1) * 128], tp[:])
        tpg = sm_ps.tile([cap, DG], BF16, tag="gps")
        nc.tensor.transpose(tpg[:], gcon[:], eye[:DG, :DG])
        nc.vector.tensor_copy(gpairT[:cap, :], tpg[:])
        if g == 0:
            nc.sync.dma_start(out=pairT[cap:cap + 1, :],
                              in_=din["bagg_row"][:])
            nc.sync.dma_start(out=gpairT[cap:cap + 1, :],
                              in_=din["bagg_g_row"][:])

        mg_ps = acc_ps.tile([P_CORE, C], F32, tag="mgps")
        nc.tensor.matmul(mg_ps[:], emat[:], pairT[:], start=True, stop=True)
        gg_ps = acc_ps.tile([P_CORE, DG], F32, tag="ggps")
        nc.tensor.matmul(gg_ps[:], emat[:], gpairT[:], start=True, stop=True)
        if g == 0:
            nc.scalar.activation(out=m_acc[:], in_=mg_ps[:], func=ACTF.Copy)
            nc.scalar.activation(out=g_acc[:], in_=gg_ps[:], func=ACTF.Copy)
        else:
            nc.vector.tensor_tensor(out=m_acc[:], in0=m_acc[:], in1=mg_ps[:],
                                    op=ALU.add)
            nc.vector.tensor_tensor(out=g_acc[:], in0=g_acc[:], in1=gg_ps[:],
                                    op=ALU.add)

    # ---- write outputs
    nc.sync.dma_start(out=m_out[:], in_=m_acc[:])
    nc.sync.dma_start(out=g_out[:], in_=g_acc[:])
    ctx.close()


# --------------------------------------------------------------------------
# entry point
# --------------------------------------------------------------------------

_PROGRAM_CACHE = {}


def _get_program(caps):
    if caps not in _PROGRAM_CACHE:
        _PROGRAM_CACHE[caps] = build_program(caps)
    return _PROGRAM_CACHE[caps]


def make_in_maps(inputs):
    pairs = _routing(inputs)
    caps = _caps_from_pairs(pairs)
    wmaps = _pack_weights(inputs)
    in_maps = []
    for core in range(N_CORES):
        m = dict(wmaps)
        m.update(_pack_core(inputs, pairs[core], caps, core))
        in_maps.append(m)
    return caps, in_maps


def _assemble(results):
    m_full = np.zeros((512, C), np.float32)
    g_full = np.zeros((512, DG), np.float32)
    for c in range(N_CORES):
        m_full[c::N_CORES] = results[c]["m_slice"]
        g_full[c::N_CORES] = results[c]["g_slice"]
    outs = []
    for s_id in range(2):
        m = m_full[s_id * 256:(s_id + 1) * 256]          # (256, C)
        outs.append(np.ascontiguousarray(
            m.reshape(H, W, C).transpose(2, 0, 1)[None]).astype(np.float32))
    for s_id in range(2):
        gsl = g_full[s_id * 256:(s_id + 1) * 256]
        outs.append(np.ascontiguousarray(
            gsl.reshape(H, W, DG).transpose(2, 0, 1)[None]).astype(np.float32))
    return tuple(outs)


def kernel(**inputs):
    caps, in_maps = make_in_maps(inputs)
    nc = _get_program(caps)
    res = run_bass_kernel_spmd(nc, in_maps, list(range(N_CORES)))
    return _assemble(res.results)


if __name__ == "__main__":
    caps = (8, 20, 28, 28, 8)
    nc = build_program(caps)
    print("built+compiled ok", caps)
